# revision 7
# baseline (speedup 1.0000x reference)
"""DEMA (double exponential smoothing) Trainium2 kernel.

x: [64, 2048, 512] fp32; recurrence over T=2048 is a 2x2 linear
time-invariant system per (batch, channel) lane:

    z_t = A z_{t-1} + B x_t,   y_t = e1^T z_t
    A = [[1-a, 1-a], [-ab, 1-ab]],  B = [a, ab]^T

Folded blocked scan: x and y are viewed as [B, T/2, 2C] so each SBUF
partition holds a PAIR of adjacent timesteps -> every DMA descriptor
moves 4KB of HBM (vs 2KB unfolded), which is what sets the wall for
this memory-bound kernel. A chunk of L=126 steps becomes 63 folded
rows; the chunk transfer matrix G is split into 4 bf16 quadrant
weights (even/odd input rows x even/odd output cols) so each chunk is
4 accumulating K<=64 matmuls per batch, using PE tile_position 64-
offsets to pack two batches per [128, 1024] tile (batch 1 at
partition 64). The (s, b) carry rides partition 0 (s in cols 0-511 of
the A-slice, b in cols 512-1023 of the B-slice) and is copied
PSUM->SBUF (cast to bf16) between chunks; carry rounding does not
compound because the recurrence is contractive (|eig| ~ 0.89).
Loads are gpsimd/SWDGE DMAs casting fp32->bf16 inline (SWDGE spreads
over all 16 SDMA engines; HWDGE only reaches 14). Stores stay fp32.
Batch dim is sharded 8 ways across cores.
"""

import sys

import numpy as np

if "/opt/trn_rl_repo" not in sys.path:
    sys.path.insert(0, "/opt/trn_rl_repo")

B, T, C = 64, 2048, 512
NCORES = 8
BPC = B // NCORES   # batches per core
L = 126             # timesteps per full chunk
NFULL = 16          # full chunks cover t = 0..2015
LT = T - NFULL * L  # tail chunk, 32 timesteps
TF = T // 2         # folded rows per batch
CF = 2 * C          # folded row width (two timesteps)
LF = L // 2         # folded rows per full chunk (63)
LTF = LT // 2       # folded tail rows (16)

NG = 4              # batch groups per core
GB = BPC // NG      # batches per group (2): batch bb at partition 64*bb

_cache = {}


def _build_mats(alpha, beta):
    """Host precompute (float64) of the folded chunk transfer blocks.

    Returns three bf16 arrays; each packs the 4 quadrant lhsT blocks
    [A_ec | A_oc | B_ec | B_oc] side by side, duplicated vertically at
    partition 64 so both PE tile positions can read them:
      g1w [128, 256]: generic chunk, K=64 rows = [carry; 63 x-pairs]
      g0w [128, 256]: chunk 0, K=63 rows = x-pairs only
      gtw [128, 64] : tail chunk, K=17 rows = [carry; 16 x-pairs]
    A-slice rows pair with rhs cols 0-511 (carry s + even timesteps),
    B-slice rows with cols 512-1023 (carry b + odd timesteps); ec/oc
    blocks produce even/odd output timesteps (out cols 0-511/512-1023).
    """
    import ml_dtypes

    a = np.float64(alpha)
    b = np.float64(beta)
    A = np.array([[1 - a, 1 - a], [-a * b, 1 - a * b]], dtype=np.float64)
    Bv = np.array([a, a * b], dtype=np.float64)
    Ap = [np.eye(2)]
    for _ in range(L + 1):
        Ap.append(Ap[-1] @ A)
    AB = np.stack([Ap[j] @ Bv for j in range(L)])  # [L, 2], A^j B
    w = AB[:, 0]                                   # w_j = e1^T A^j B

    # Generic chunk starting at t0, carry z_{t0-1} in rows 0-1:
    #   z_{t0+tau} = A^{tau+1} z_{t0-1} + sum_k A^{tau-k} B x_{t0+k}
    # cols: 0-1 chunk-end state, 2+tau -> y_{t0+tau}
    G1 = np.zeros((128, 128))
    for tau in range(L):
        m = 2 + tau
        G1[0, m] = Ap[tau + 1][0, 0]
        G1[1, m] = Ap[tau + 1][0, 1]
        for k in range(tau + 1):
            G1[2 + k, m] = w[tau - k]
    for j in range(2):
        for jp in range(2):
            G1[j, jp] = Ap[L][jp, j]
    for k in range(L):
        G1[2 + k, 0] = AB[L - 1 - k][0]
        G1[2 + k, 1] = AB[L - 1 - k][1]

    # Chunk 0: z_0 = (x_0, x_1 - x_0), y_0 = x_0; no carry rows.
    G0 = np.zeros((128, 128))
    G0[2, 2] = 1.0
    for tau in range(1, L):
        m = 2 + tau
        G0[2, m] = Ap[tau][0, 0] - Ap[tau][0, 1]
        G0[3, m] = Ap[tau][0, 1] + w[tau - 1]
        for k in range(2, tau + 1):
            G0[2 + k, m] = w[tau - k]
    for jp in range(2):
        G0[2, jp] = Ap[L - 1][jp, 0] - Ap[L - 1][jp, 1]
        G0[3, jp] = Ap[L - 1][jp, 1] + AB[L - 2][jp]
        for k in range(2, L):
            G0[2 + k, jp] = AB[L - 1 - k][jp]
    G0x = G0[2:128]  # [126, 128] x rows only

    # Tail chunk: carry rows 0-1, LT output cols (no state cols).
    Gt = np.zeros((2 + LT, LT))
    for tau in range(LT):
        Gt[0, tau] = Ap[tau + 1][0, 0]
        Gt[1, tau] = Ap[tau + 1][0, 1]
        for k in range(tau + 1):
            Gt[2 + k, tau] = w[tau - k]

    bf16 = ml_dtypes.bfloat16

    def pack(blocks, rows):
        blk = np.hstack(blocks)                   # [rows, 4*M]
        out = np.zeros((128, blk.shape[1]), dtype=np.float64)
        out[0:rows] = blk
        out[64:64 + rows] = blk
        return out.astype(bf16)

    def fold_carry(G):   # carry rows 0-1, x rows 2.. ; all cols folded
        Ar = np.vstack([G[0:1], G[2::2]])
        Br = np.vstack([G[1:2], G[3::2]])
        return [Ar[:, 0::2], Ar[:, 1::2], Br[:, 0::2], Br[:, 1::2]]

    g1w = pack(fold_carry(G1), 64)                          # [128, 256]
    g0w = pack([G0x[0::2, 0::2], G0x[0::2, 1::2],
                G0x[1::2, 0::2], G0x[1::2, 1::2]], LF)      # [128, 256]
    gtw = pack(fold_carry(Gt), 1 + LTF)                     # [128, 64]
    return g0w, g1w, gtw


def _build_program():
    import concourse.mybir as mybir
    import concourse.tile as tile
    from concourse import bacc

    FP32 = mybir.dt.float32
    BF16 = mybir.dt.bfloat16
    nc = bacc.Bacc(
        "TRN2", target_bir_lowering=False, debug=False, enable_asserts=False
    )
    x_d = nc.dram_tensor("x", [BPC, TF, CF], FP32, kind="ExternalInput").ap()
    g0_d = nc.dram_tensor("g0", [128, 256], BF16, kind="ExternalInput").ap()
    g1_d = nc.dram_tensor("g1", [128, 256], BF16, kind="ExternalInput").ap()
    gt_d = nc.dram_tensor("gt", [128, 64], BF16, kind="ExternalInput").ap()
    y_d = nc.dram_tensor("y", [BPC, TF, CF], FP32, kind="ExternalOutput").ap()

    with tile.TileContext(nc) as tc:
        with (
            tc.tile_pool(name="g", bufs=1) as gpool,
            tc.tile_pool(name="xp", bufs=12) as xpool,
            tc.tile_pool(name="op", bufs=10) as opool,
            tc.tile_pool(name="ps", bufs=4, space="PSUM") as pspool,
        ):
            g0 = gpool.tile([128, 256], BF16, tag="g0")
            g1 = gpool.tile([128, 256], BF16, tag="g1")
            gt = gpool.tile([128, 64], BF16, tag="gt")
            nc.scalar.dma_start(out=g0[:], in_=g0_d)
            nc.scalar.dma_start(out=g1[:], in_=g1_d)
            nc.scalar.dma_start(out=gt[:], in_=gt_d)

            # round-0 inputs: x-pairs at partitions q..q+62 (no carry row;
            # K=63 keeps the PE tile base at 0/64). Casting loads must ride
            # gpsimd (SWDGE) - which also spreads descriptors over all 16
            # SDMA engines instead of HWDGE's 14.
            xcur = []
            for g in range(NG):
                xs = xpool.tile([128, CF], BF16, tag="x")
                for bb in range(GB):
                    b = g * GB + bb
                    q = 64 * bb
                    nc.gpsimd.dma_start(
                        out=xs[q:q + LF, :], in_=x_d[b, 0:LF, :]
                    )
                xcur.append(xs)

            for i in range(NFULL + 1):
                for g in range(NG):
                    xs = xcur[g]
                    ps = pspool.tile([128, CF], FP32, tag="ps")
                    if i < NFULL:
                        # prefetch next round's x-pairs before this round's
                        # matmuls so the DMA queues never starve
                        nxt = xpool.tile([128, CF], BF16, tag="x")
                        nrows = LF if i + 1 < NFULL else LTF
                        for bb in range(GB):
                            b = g * GB + bb
                            q = 64 * bb
                            nc.gpsimd.dma_start(
                                out=nxt[q + 1:q + 1 + nrows, :],
                                in_=x_d[b, LF * (i + 1):LF * (i + 1) + nrows, :],
                            )
                        for bb in range(GB):
                            q = 64 * bb
                            if i == 0:
                                lw, k0, k1 = g0, q, q + LF
                            else:
                                lw, k0, k1 = g1, q, q + 64
                            rA = xs[k0:k1, 0:C]
                            rB = xs[k0:k1, C:CF]
                            pe = ps[q:q + 64, 0:C]
                            po = ps[q:q + 64, C:CF]
                            nc.tensor.matmul(pe, lw[k0:k1, 0:64], rA,
                                             start=True, stop=False)
                            nc.tensor.matmul(pe, lw[k0:k1, 128:192], rB,
                                             start=False, stop=True)
                            nc.tensor.matmul(po, lw[k0:k1, 64:128], rA,
                                             start=True, stop=False)
                            nc.tensor.matmul(po, lw[k0:k1, 192:256], rB,
                                             start=False, stop=True)
                        # chunk-end states (partition q, s|b) -> next rhs
                        for bb in range(GB):
                            q = 64 * bb
                            nc.vector.tensor_copy(
                                out=nxt[q:q + 1, :], in_=ps[q:q + 1, :]
                            )
                        xcur[g] = nxt
                        o = opool.tile([128, CF], FP32, tag="o")
                        nc.scalar.copy(out=o[:], in_=ps[:])
                        for bb in range(GB):
                            b = g * GB + bb
                            q = 64 * bb
                            # near the kernel tail, spread out-DMA issue
                            # over the otherwise-idle HWDGE engines
                            if i >= NFULL - 1:
                                eng = (nc.gpsimd, nc.sync)[(g * GB + bb) % 2]
                            else:
                                eng = nc.gpsimd
                            eng.dma_start(
                                out=y_d[b, LF * i:LF * (i + 1), :],
                                in_=o[q + 1:q + 64, :],
                            )
                    else:  # tail chunk: 16 folded rows, no state outputs
                        for bb in range(GB):
                            q = 64 * bb
                            rA = xs[q:q + 1 + LTF, 0:C]
                            rB = xs[q:q + 1 + LTF, C:CF]
                            pe = ps[q:q + LTF, 0:C]
                            po = ps[q:q + LTF, C:CF]
                            nc.tensor.matmul(pe, gt[q:q + 1 + LTF, 0:16], rA,
                                             start=True, stop=False)
                            nc.tensor.matmul(pe, gt[q:q + 1 + LTF, 32:48], rB,
                                             start=False, stop=True)
                            nc.tensor.matmul(po, gt[q:q + 1 + LTF, 16:32], rA,
                                             start=True, stop=False)
                            nc.tensor.matmul(po, gt[q:q + 1 + LTF, 48:64], rB,
                                             start=False, stop=True)
                        o = opool.tile([128, CF], FP32, tag="o")
                        for bb in range(GB):
                            q = 64 * bb
                            nc.scalar.copy(
                                out=o[q:q + LTF, :], in_=ps[q:q + LTF, :]
                            )
                        for bb in range(GB):
                            b = g * GB + bb
                            q = 64 * bb
                            eng = (nc.gpsimd, nc.sync, nc.scalar)[(g * GB + bb) % 3]
                            eng.dma_start(
                                out=y_d[b, LF * NFULL:TF, :],
                                in_=o[q:q + LTF, :],
                            )
    nc.compile()
    return nc


def _get_program():
    if "nc" not in _cache:
        _cache["nc"] = _build_program()
    return _cache["nc"]


def _ensure_axon_hooks_shim():
    """concourse's trace path does `from antenv.axon_hooks import ...`;
    some images lack that module. Install a no-op shim so an externally
    set BASS_TRACE can't crash the run (tracing then degrades to off)."""
    import types

    try:
        import antenv.axon_hooks  # noqa: F401
        return
    except ImportError:
        pass
    try:
        import antenv
    except ImportError:
        return
    mod = types.ModuleType("antenv.axon_hooks")
    mod.get_axon_ntff_profile_hook = lambda: None
    mod.set_axon_ntff_profile_hook = lambda h: None
    mod._kernel_shim = True
    sys.modules["antenv.axon_hooks"] = mod
    antenv.axon_hooks = mod


def _run(x, alpha, beta, trace=False):
    _ensure_axon_hooks_shim()
    from concourse.bass_utils import run_bass_kernel_spmd

    x = np.ascontiguousarray(np.asarray(x, dtype=np.float32))
    xf = x.reshape(B, TF, CF)
    G0, G1, Gt = _build_mats(alpha, beta)
    nc = _get_program()
    in_maps = [
        {"x": xf[c * BPC:(c + 1) * BPC], "g0": G0, "g1": G1, "gt": Gt}
        for c in range(NCORES)
    ]
    res = run_bass_kernel_spmd(nc, in_maps, list(range(NCORES)), trace=trace)
    out = np.concatenate([res.results[c]["y"] for c in range(NCORES)], axis=0)
    return out.reshape(B, T, C), res


def kernel(**inputs):
    alpha = float(np.asarray(inputs["alpha"]))
    beta = float(np.asarray(inputs["beta"]))
    out, _ = _run(inputs["x"], alpha, beta, trace=False)
    return out


# revision 9
# speedup vs baseline: 1.1142x; 1.1142x over previous
"""DEMA (double exponential smoothing) Trainium2 kernel.

x: [64, 2048, 512] fp32; recurrence over T=2048 is a 2x2 linear
time-invariant system per (batch, channel) lane:

    z_t = A z_{t-1} + B x_t,   y_t = e1^T z_t
    A = [[1-a, 1-a], [-ab, 1-ab]],  B = [a, ab]^T

Blocked scan: chunks of L=126 timesteps. One [128x128] @ [128x512]
fp32 matmul per (batch, chunk): rhs rows 0-1 carry the (s, b) state
into the chunk, rows 2..127 carry the chunk's inputs; lhsT columns
0-1 produce the chunk-end state (fed into the next chunk's rhs rows
0-1 via a tiny PSUM->SBUF copy), columns 2..127 produce the outputs.
Batch dim is sharded 8 ways across cores; within a core the 8
batches' chunk chains are interleaved chunk-major so the PE always
has independent work while each carry chain advances.
"""

import sys

import numpy as np

if "/opt/trn_rl_repo" not in sys.path:
    sys.path.insert(0, "/opt/trn_rl_repo")

B, T, C = 64, 2048, 512
NCORES = 8
BPC = B // NCORES  # batches per core
L = 126            # timesteps per full chunk (126 outputs + 2 state cols = 128)
NFULL = 16         # full chunks cover t = 0..2015
LT = T - NFULL * L  # tail chunk, 32 timesteps

_cache = {}


def _build_mats(alpha, beta):
    """Per-call host precompute of the chunk transfer matrices (float64)."""
    a = np.float64(alpha)
    b = np.float64(beta)
    A = np.array([[1 - a, 1 - a], [-a * b, 1 - a * b]], dtype=np.float64)
    Bv = np.array([a, a * b], dtype=np.float64)
    Ap = [np.eye(2)]
    for _ in range(L):
        Ap.append(Ap[-1] @ A)
    AB = np.stack([Ap[j] @ Bv for j in range(L)])  # [L, 2], A^j B
    w = AB[:, 0]                                   # w_j = e1^T A^j B

    # Generic chunk starting at t0, carry z_{t0-1} in rhs rows 0-1:
    #   z_{t0+tau} = A^{tau+1} z_{t0-1} + sum_k A^{tau-k} B x_{t0+k}
    G1 = np.zeros((128, 128))
    for tau in range(L):
        m = 2 + tau
        G1[0, m] = Ap[tau + 1][0, 0]
        G1[1, m] = Ap[tau + 1][0, 1]
        for k in range(tau + 1):
            G1[2 + k, m] = w[tau - k]
    for j in range(2):
        for jp in range(2):
            G1[j, jp] = Ap[L][jp, j]
    for k in range(L):
        G1[2 + k, 0] = AB[L - 1 - k][0]
        G1[2 + k, 1] = AB[L - 1 - k][1]

    # Chunk 0: z_0 = (x_0, x_1 - x_0), y_0 = x_0, rhs rows 0-1 are zero.
    G0 = np.zeros((128, 128))
    G0[2, 2] = 1.0
    for tau in range(1, L):
        m = 2 + tau
        G0[2, m] = Ap[tau][0, 0] - Ap[tau][0, 1]
        G0[3, m] = Ap[tau][0, 1] + w[tau - 1]
        for k in range(2, tau + 1):
            G0[2 + k, m] = w[tau - k]
    for jp in range(2):
        G0[2, jp] = Ap[L - 1][jp, 0] - Ap[L - 1][jp, 1]
        G0[3, jp] = Ap[L - 1][jp, 1] + AB[L - 2][jp]
        for k in range(2, L):
            G0[2 + k, jp] = AB[L - 1 - k][jp]

    # Tail chunk: LT outputs, no state columns.
    Gt = np.zeros((2 + LT, LT))
    for tau in range(LT):
        Gt[0, tau] = Ap[tau + 1][0, 0]
        Gt[1, tau] = Ap[tau + 1][0, 1]
        for k in range(tau + 1):
            Gt[2 + k, tau] = w[tau - k]
    # drop G0's all-zero carry rows: round 0's rhs has inputs at rows 0..L-1
    return (
        G0[2:128].astype(np.float32),
        G1.astype(np.float32),
        Gt.astype(np.float32),
    )


NG = 4             # batch groups per core
GB = BPC // NG     # batches per group (2) -> group tiles are [*, GB*C]


def _build_program():
    import concourse.mybir as mybir
    import concourse.tile as tile
    from concourse import bacc

    FP32 = mybir.dt.float32
    W = GB * C  # group tile width in the free dim
    nc = bacc.Bacc(
        "TRN2", target_bir_lowering=False, debug=False, enable_asserts=False
    )
    x_d = nc.dram_tensor("x", [BPC, T, C], FP32, kind="ExternalInput").ap()
    # g0 is the chunk-0 matrix with the (all-zero) carry rows dropped:
    # [126, 128], so round 0's rhs needs no zeroed carry rows.
    g0_d = nc.dram_tensor("g0", [L, 128], FP32, kind="ExternalInput").ap()
    g1_d = nc.dram_tensor("g1", [128, 128], FP32, kind="ExternalInput").ap()
    gt_d = nc.dram_tensor("gt", [2 + LT, LT], FP32, kind="ExternalInput").ap()
    y_d = nc.dram_tensor("y", [BPC, T, C], FP32, kind="ExternalOutput").ap()

    with tile.TileContext(nc) as tc:
        with (
            tc.tile_pool(name="g", bufs=1) as gpool,
            tc.tile_pool(name="xp", bufs=14) as xpool,
            tc.tile_pool(name="op", bufs=12) as opool,
            tc.tile_pool(name="ps", bufs=4, space="PSUM") as pspool,
        ):
            g0 = gpool.tile([L, 128], FP32, tag="g0")
            g1 = gpool.tile([128, 128], FP32, tag="g1")
            gt = gpool.tile([2 + LT, LT], FP32, tag="gt")
            nc.scalar.dma_start(out=g0[:], in_=g0_d)

            # round-0 inputs: rows 0..L-1 (no carry rows); split the issue
            # across both HWDGE engines to halve prologue latency. g1/gt
            # load after (g1 is first needed a full round later).
            xcur = []
            for g in range(NG):
                xs = xpool.tile([L, W], FP32, tag="x")
                for bb in range(GB):
                    b = g * GB + bb
                    eng = nc.sync if (g * GB + bb) % 2 == 0 else nc.scalar
                    eng.dma_start(
                        out=xs[0:L, bb * C:(bb + 1) * C], in_=x_d[b, 0:L, :]
                    )
                xcur.append(xs)
            nc.scalar.dma_start(out=g1[:], in_=g1_d)
            nc.scalar.dma_start(out=gt[:], in_=gt_d)

            ncopy = 0
            for i in range(NFULL + 1):
                for g in range(NG):
                    xs = xcur[g]
                    ps = pspool.tile([128, W], FP32, tag="ps")
                    if i < NFULL:
                        # next round's input tile for this group; issue the
                        # prefetch DMAs before this round's matmuls so the
                        # DMA queues never starve
                        if i + 1 < NFULL:
                            nxt = xpool.tile([128, W], FP32, tag="x")
                            nrows = L
                        else:
                            nxt = xpool.tile([2 + LT, W], FP32, tag="x")
                            nrows = LT
                        for bb in range(GB):
                            b = g * GB + bb
                            nc.sync.dma_start(
                                out=nxt[2:2 + nrows, bb * C:(bb + 1) * C],
                                in_=x_d[b, L * (i + 1):L * (i + 1) + nrows, :],
                            )
                        for bb in range(GB):
                            sl = slice(bb * C, (bb + 1) * C)
                            if i == 0:
                                nc.tensor.matmul(
                                    ps[:, sl], g0[:], xs[0:L, sl],
                                    start=True, stop=True,
                                )
                            else:
                                nc.tensor.matmul(
                                    ps[:, sl], g1[:], xs[:, sl],
                                    start=True, stop=True,
                                )
                        # chunk-end states -> next rhs rows 0-1
                        nc.vector.tensor_copy(out=nxt[0:2, :], in_=ps[0:2, :])
                        xcur[g] = nxt
                        o = opool.tile([128, W], FP32, tag="o")
                        nc.scalar.copy(out=o[:], in_=ps[:])
                        for bb in range(GB):
                            b = g * GB + bb
                            # near the kernel tail, spread out-DMA issue over
                            # the otherwise-idle DMA engines
                            if i >= NFULL - 2:
                                eng = (nc.gpsimd, nc.sync)[(g * GB + bb) % 2]
                            else:
                                eng = nc.gpsimd
                            eng.dma_start(
                                out=y_d[b, L * i:L * (i + 1), :],
                                in_=o[2:128, bb * C:(bb + 1) * C],
                            )
                    else:  # tail chunk (32 steps, no state outputs)
                        for bb in range(GB):
                            sl = slice(bb * C, (bb + 1) * C)
                            nc.tensor.matmul(
                                ps[0:LT, sl], gt[:], xs[0:2 + LT, sl],
                                start=True, stop=True,
                            )
                        o = opool.tile([LT, W], FP32, tag="o")
                        nc.scalar.copy(out=o[:], in_=ps[0:LT, :])
                        for bb in range(GB):
                            b = g * GB + bb
                            eng = (nc.gpsimd, nc.sync, nc.scalar)[(g * GB + bb) % 3]
                            eng.dma_start(
                                out=y_d[b, L * NFULL:T, :],
                                in_=o[:, bb * C:(bb + 1) * C],
                            )
    nc.compile()
    return nc


def _get_program():
    if "nc" not in _cache:
        _cache["nc"] = _build_program()
    return _cache["nc"]


def _ensure_axon_hooks_shim():
    """concourse's trace path does `from antenv.axon_hooks import ...`;
    some images lack that module. Install a no-op shim so an externally
    set BASS_TRACE can't crash the run (tracing then degrades to off)."""
    import types

    try:
        import antenv.axon_hooks  # noqa: F401
        return
    except ImportError:
        pass
    try:
        import antenv
    except ImportError:
        return
    mod = types.ModuleType("antenv.axon_hooks")
    mod.get_axon_ntff_profile_hook = lambda: None
    mod.set_axon_ntff_profile_hook = lambda h: None
    mod._kernel_shim = True
    sys.modules["antenv.axon_hooks"] = mod
    antenv.axon_hooks = mod


def _run(x, alpha, beta, trace=False):
    _ensure_axon_hooks_shim()
    from concourse.bass_utils import run_bass_kernel_spmd

    x = np.ascontiguousarray(np.asarray(x, dtype=np.float32))
    G0, G1, Gt = _build_mats(alpha, beta)
    nc = _get_program()
    in_maps = [
        {"x": x[c * BPC:(c + 1) * BPC], "g0": G0, "g1": G1, "gt": Gt}
        for c in range(NCORES)
    ]
    res = run_bass_kernel_spmd(nc, in_maps, list(range(NCORES)), trace=trace)
    out = np.concatenate([res.results[c]["y"] for c in range(NCORES)], axis=0)
    return out, res


def kernel(**inputs):
    alpha = float(np.asarray(inputs["alpha"]))
    beta = float(np.asarray(inputs["beta"]))
    out, _ = _run(inputs["x"], alpha, beta, trace=False)
    return out



# revision 10
# speedup vs baseline: 1.2016x; 1.0785x over previous
"""DEMA (double exponential smoothing) Trainium2 kernel.

x: [64, 2048, 512] fp32; recurrence over T=2048 is a 2x2 linear
time-invariant system per (batch, channel) lane:

    z_t = A z_{t-1} + B x_t,   y_t = e1^T z_t
    A = [[1-a, 1-a], [-ab, 1-ab]],  B = [a, ab]^T

Blocked scan: chunks of L=126 timesteps. One [128x128] @ [128x512]
fp32 matmul per (batch, chunk): rhs rows 0-1 carry the (s, b) state
into the chunk, rows 2..127 carry the chunk's inputs; lhsT columns
0-1 produce the chunk-end state (fed into the next chunk's rhs rows
0-1 via a tiny PSUM->SBUF copy), columns 2..127 produce the outputs.
Batch dim is sharded 8 ways across cores; within a core the 8
batches' chunk chains are interleaved chunk-major so the PE always
has independent work while each carry chain advances.
"""

import sys

import numpy as np

if "/opt/trn_rl_repo" not in sys.path:
    sys.path.insert(0, "/opt/trn_rl_repo")

B, T, C = 64, 2048, 512
NCORES = 8
BPC = B // NCORES  # batches per core
L = 126            # timesteps per full chunk (126 outputs + 2 state cols = 128)
NFULL = 16         # full chunks cover t = 0..2015
LT = T - NFULL * L  # tail chunk, 32 timesteps

_cache = {}


def _build_mats(alpha, beta):
    """Per-call host precompute of the chunk transfer matrices (float64)."""
    a = np.float64(alpha)
    b = np.float64(beta)
    A = np.array([[1 - a, 1 - a], [-a * b, 1 - a * b]], dtype=np.float64)
    Bv = np.array([a, a * b], dtype=np.float64)
    Ap = [np.eye(2)]
    for _ in range(L):
        Ap.append(Ap[-1] @ A)
    AB = np.stack([Ap[j] @ Bv for j in range(L)])  # [L, 2], A^j B
    w = AB[:, 0]                                   # w_j = e1^T A^j B

    # Generic chunk starting at t0, carry z_{t0-1} in rhs rows 0-1:
    #   z_{t0+tau} = A^{tau+1} z_{t0-1} + sum_k A^{tau-k} B x_{t0+k}
    G1 = np.zeros((128, 128))
    for tau in range(L):
        m = 2 + tau
        G1[0, m] = Ap[tau + 1][0, 0]
        G1[1, m] = Ap[tau + 1][0, 1]
        for k in range(tau + 1):
            G1[2 + k, m] = w[tau - k]
    for j in range(2):
        for jp in range(2):
            G1[j, jp] = Ap[L][jp, j]
    for k in range(L):
        G1[2 + k, 0] = AB[L - 1 - k][0]
        G1[2 + k, 1] = AB[L - 1 - k][1]

    # Chunk 0: z_0 = (x_0, x_1 - x_0), y_0 = x_0, rhs rows 0-1 are zero.
    G0 = np.zeros((128, 128))
    G0[2, 2] = 1.0
    for tau in range(1, L):
        m = 2 + tau
        G0[2, m] = Ap[tau][0, 0] - Ap[tau][0, 1]
        G0[3, m] = Ap[tau][0, 1] + w[tau - 1]
        for k in range(2, tau + 1):
            G0[2 + k, m] = w[tau - k]
    for jp in range(2):
        G0[2, jp] = Ap[L - 1][jp, 0] - Ap[L - 1][jp, 1]
        G0[3, jp] = Ap[L - 1][jp, 1] + AB[L - 2][jp]
        for k in range(2, L):
            G0[2 + k, jp] = AB[L - 1 - k][jp]

    # Tail chunk: LT outputs, no state columns.
    Gt = np.zeros((2 + LT, LT))
    for tau in range(LT):
        Gt[0, tau] = Ap[tau + 1][0, 0]
        Gt[1, tau] = Ap[tau + 1][0, 1]
        for k in range(tau + 1):
            Gt[2 + k, tau] = w[tau - k]
    # drop G0's all-zero carry rows: round 0's rhs has inputs at rows 0..L-1
    return (
        G0[2:128].astype(np.float32),
        G1.astype(np.float32),
        Gt.astype(np.float32),
    )


NG = 4             # batch groups per core
GB = BPC // NG     # batches per group (2) -> group tiles are [*, GB*C]


def _build_program():
    import concourse.mybir as mybir
    import concourse.tile as tile
    from concourse import bacc

    FP32 = mybir.dt.float32
    W = GB * C  # group tile width in the free dim
    nc = bacc.Bacc(
        "TRN2", target_bir_lowering=False, debug=False, enable_asserts=False
    )
    x_d = nc.dram_tensor("x", [BPC, T, C], FP32, kind="ExternalInput").ap()
    # g0 is the chunk-0 matrix with the (all-zero) carry rows dropped:
    # [126, 128], so round 0's rhs needs no zeroed carry rows.
    g0_d = nc.dram_tensor("g0", [L, 128], FP32, kind="ExternalInput").ap()
    g1_d = nc.dram_tensor("g1", [128, 128], FP32, kind="ExternalInput").ap()
    gt_d = nc.dram_tensor("gt", [2 + LT, LT], FP32, kind="ExternalInput").ap()
    y_d = nc.dram_tensor("y", [BPC, T, C], FP32, kind="ExternalOutput").ap()

    with tile.TileContext(nc) as tc:
        with (
            tc.tile_pool(name="g", bufs=1) as gpool,
            tc.tile_pool(name="xp", bufs=14) as xpool,
            tc.tile_pool(name="op", bufs=12) as opool,
            tc.tile_pool(name="ps", bufs=4, space="PSUM") as pspool,
        ):
            g0 = gpool.tile([L, 128], FP32, tag="g0")
            g1 = gpool.tile([128, 128], FP32, tag="g1")
            gt = gpool.tile([2 + LT, LT], FP32, tag="gt")
            nc.scalar.dma_start(out=g0[:], in_=g0_d)

            # round-0 inputs: rows 0..L-1 (no carry rows); split the issue
            # across both HWDGE engines to halve prologue latency. g1/gt
            # load after (g1 is first needed a full round later).
            xcur = []
            for g in range(NG):
                xs = xpool.tile([L, W], FP32, tag="x")
                for bb in range(GB):
                    b = g * GB + bb
                    eng = nc.sync if (g * GB + bb) % 2 == 0 else nc.scalar
                    eng.dma_start(
                        out=xs[0:L, bb * C:(bb + 1) * C], in_=x_d[b, 0:L, :]
                    )
                xcur.append(xs)
            nc.scalar.dma_start(out=g1[:], in_=g1_d)
            nc.scalar.dma_start(out=gt[:], in_=gt_d)

            ncopy = 0
            for i in range(NFULL + 1):
                for g in range(NG):
                    xs = xcur[g]
                    ps = pspool.tile([128, W], FP32, tag="ps")
                    if i < NFULL:
                        # next round's input tile for this group; issue the
                        # prefetch DMAs before this round's matmuls so the
                        # DMA queues never starve
                        if i + 1 < NFULL:
                            nxt = xpool.tile([128, W], FP32, tag="x")
                            nrows = L
                        else:
                            nxt = xpool.tile([2 + LT, W], FP32, tag="x")
                            nrows = LT
                        for bb in range(GB):
                            b = g * GB + bb
                            nc.sync.dma_start(
                                out=nxt[2:2 + nrows, bb * C:(bb + 1) * C],
                                in_=x_d[b, L * (i + 1):L * (i + 1) + nrows, :],
                            )
                        for bb in range(GB):
                            sl = slice(bb * C, (bb + 1) * C)
                            if i == 0:
                                nc.tensor.matmul(
                                    ps[:, sl], g0[:], xs[0:L, sl],
                                    start=True, stop=True,
                                )
                            else:
                                nc.tensor.matmul(
                                    ps[:, sl], g1[:], xs[:, sl],
                                    start=True, stop=True,
                                )
                        # chunk-end states -> next rhs rows 0-1
                        nc.vector.tensor_copy(out=nxt[0:2, :], in_=ps[0:2, :])
                        xcur[g] = nxt
                        o = opool.tile([128, W], FP32, tag="o")
                        nc.scalar.copy(out=o[:], in_=ps[:])
                        for bb in range(GB):
                            b = g * GB + bb
                            # near the kernel tail, spread out-DMA issue over
                            # the otherwise-idle DMA engines
                            if i >= NFULL - 1:
                                eng = (nc.gpsimd, nc.sync)[(g * GB + bb) % 2]
                            else:
                                eng = nc.gpsimd
                            eng.dma_start(
                                out=y_d[b, L * i:L * (i + 1), :],
                                in_=o[2:128, bb * C:(bb + 1) * C],
                            )
                    else:  # tail chunk (32 steps, no state outputs)
                        for bb in range(GB):
                            sl = slice(bb * C, (bb + 1) * C)
                            nc.tensor.matmul(
                                ps[0:LT, sl], gt[:], xs[0:2 + LT, sl],
                                start=True, stop=True,
                            )
                        o = opool.tile([LT, W], FP32, tag="o")
                        nc.scalar.copy(out=o[:], in_=ps[0:LT, :])
                        for bb in range(GB):
                            b = g * GB + bb
                            eng = (nc.gpsimd, nc.sync, nc.scalar)[(g * GB + bb) % 3]
                            eng.dma_start(
                                out=y_d[b, L * NFULL:T, :],
                                in_=o[:, bb * C:(bb + 1) * C],
                            )
    nc.compile()
    return nc


def _get_program():
    if "nc" not in _cache:
        _cache["nc"] = _build_program()
    return _cache["nc"]


def _ensure_axon_hooks_shim():
    """concourse's trace path does `from antenv.axon_hooks import ...`;
    some images lack that module. Install a no-op shim so an externally
    set BASS_TRACE can't crash the run (tracing then degrades to off)."""
    import types

    try:
        import antenv.axon_hooks  # noqa: F401
        return
    except ImportError:
        pass
    try:
        import antenv
    except ImportError:
        return
    mod = types.ModuleType("antenv.axon_hooks")
    mod.get_axon_ntff_profile_hook = lambda: None
    mod.set_axon_ntff_profile_hook = lambda h: None
    mod._kernel_shim = True
    sys.modules["antenv.axon_hooks"] = mod
    antenv.axon_hooks = mod


def _run(x, alpha, beta, trace=False):
    _ensure_axon_hooks_shim()
    from concourse.bass_utils import run_bass_kernel_spmd

    x = np.ascontiguousarray(np.asarray(x, dtype=np.float32))
    G0, G1, Gt = _build_mats(alpha, beta)
    nc = _get_program()
    in_maps = [
        {"x": x[c * BPC:(c + 1) * BPC], "g0": G0, "g1": G1, "gt": Gt}
        for c in range(NCORES)
    ]
    res = run_bass_kernel_spmd(nc, in_maps, list(range(NCORES)), trace=trace)
    out = np.concatenate([res.results[c]["y"] for c in range(NCORES)], axis=0)
    return out, res


def kernel(**inputs):
    alpha = float(np.asarray(inputs["alpha"]))
    beta = float(np.asarray(inputs["beta"]))
    out, _ = _run(inputs["x"], alpha, beta, trace=False)
    return out



# revision 11
# speedup vs baseline: 1.2048x; 1.0027x over previous
"""DEMA (double exponential smoothing) Trainium2 kernel.

x: [64, 2048, 512] fp32; recurrence over T=2048 is a 2x2 linear
time-invariant system per (batch, channel) lane:

    z_t = A z_{t-1} + B x_t,   y_t = e1^T z_t
    A = [[1-a, 1-a], [-ab, 1-ab]],  B = [a, ab]^T

Blocked scan: chunks of L=126 timesteps. One [128x128] @ [128x512]
fp32 matmul per (batch, chunk): rhs rows 0-1 carry the (s, b) state
into the chunk, rows 2..127 carry the chunk's inputs; lhsT columns
0-1 produce the chunk-end state (fed into the next chunk's rhs rows
0-1 via a tiny PSUM->SBUF copy), columns 2..127 produce the outputs.
Batch dim is sharded 8 ways across cores; within a core the 8
batches' chunk chains are interleaved chunk-major so the PE always
has independent work while each carry chain advances.
"""

import sys

import numpy as np

if "/opt/trn_rl_repo" not in sys.path:
    sys.path.insert(0, "/opt/trn_rl_repo")

B, T, C = 64, 2048, 512
NCORES = 8
BPC = B // NCORES  # batches per core
L = 126            # timesteps per full chunk (126 outputs + 2 state cols = 128)
NFULL = 16         # full chunks cover t = 0..2015
LT = T - NFULL * L  # tail chunk, 32 timesteps

_cache = {}


def _build_mats(alpha, beta):
    """Per-call host precompute of the chunk transfer matrices (float64)."""
    a = np.float64(alpha)
    b = np.float64(beta)
    A = np.array([[1 - a, 1 - a], [-a * b, 1 - a * b]], dtype=np.float64)
    Bv = np.array([a, a * b], dtype=np.float64)
    Ap = [np.eye(2)]
    for _ in range(L):
        Ap.append(Ap[-1] @ A)
    AB = np.stack([Ap[j] @ Bv for j in range(L)])  # [L, 2], A^j B
    w = AB[:, 0]                                   # w_j = e1^T A^j B

    # Generic chunk starting at t0, carry z_{t0-1} in rhs rows 0-1:
    #   z_{t0+tau} = A^{tau+1} z_{t0-1} + sum_k A^{tau-k} B x_{t0+k}
    G1 = np.zeros((128, 128))
    for tau in range(L):
        m = 2 + tau
        G1[0, m] = Ap[tau + 1][0, 0]
        G1[1, m] = Ap[tau + 1][0, 1]
        for k in range(tau + 1):
            G1[2 + k, m] = w[tau - k]
    for j in range(2):
        for jp in range(2):
            G1[j, jp] = Ap[L][jp, j]
    for k in range(L):
        G1[2 + k, 0] = AB[L - 1 - k][0]
        G1[2 + k, 1] = AB[L - 1 - k][1]

    # Chunk 0: z_0 = (x_0, x_1 - x_0), y_0 = x_0, rhs rows 0-1 are zero.
    G0 = np.zeros((128, 128))
    G0[2, 2] = 1.0
    for tau in range(1, L):
        m = 2 + tau
        G0[2, m] = Ap[tau][0, 0] - Ap[tau][0, 1]
        G0[3, m] = Ap[tau][0, 1] + w[tau - 1]
        for k in range(2, tau + 1):
            G0[2 + k, m] = w[tau - k]
    for jp in range(2):
        G0[2, jp] = Ap[L - 1][jp, 0] - Ap[L - 1][jp, 1]
        G0[3, jp] = Ap[L - 1][jp, 1] + AB[L - 2][jp]
        for k in range(2, L):
            G0[2 + k, jp] = AB[L - 1 - k][jp]

    # Tail chunk: LT outputs, no state columns.
    Gt = np.zeros((2 + LT, LT))
    for tau in range(LT):
        Gt[0, tau] = Ap[tau + 1][0, 0]
        Gt[1, tau] = Ap[tau + 1][0, 1]
        for k in range(tau + 1):
            Gt[2 + k, tau] = w[tau - k]
    # drop G0's all-zero carry rows: round 0's rhs has inputs at rows 0..L-1
    return (
        G0[2:128].astype(np.float32),
        G1.astype(np.float32),
        Gt.astype(np.float32),
    )


NG = 4             # batch groups per core
GB = BPC // NG     # batches per group (2) -> group tiles are [*, GB*C]


def _build_program():
    import concourse.mybir as mybir
    import concourse.tile as tile
    from concourse import bacc

    FP32 = mybir.dt.float32
    W = GB * C  # group tile width in the free dim
    nc = bacc.Bacc(
        "TRN2", target_bir_lowering=False, debug=False, enable_asserts=False
    )
    x_d = nc.dram_tensor("x", [BPC, T, C], FP32, kind="ExternalInput").ap()
    # g0 is the chunk-0 matrix with the (all-zero) carry rows dropped:
    # [126, 128], so round 0's rhs needs no zeroed carry rows.
    g0_d = nc.dram_tensor("g0", [L, 128], FP32, kind="ExternalInput").ap()
    g1_d = nc.dram_tensor("g1", [128, 128], FP32, kind="ExternalInput").ap()
    gt_d = nc.dram_tensor("gt", [2 + LT, LT], FP32, kind="ExternalInput").ap()
    y_d = nc.dram_tensor("y", [BPC, T, C], FP32, kind="ExternalOutput").ap()

    with tile.TileContext(nc) as tc:
        with (
            tc.tile_pool(name="g", bufs=1) as gpool,
            tc.tile_pool(name="xp", bufs=12) as xpool,
            tc.tile_pool(name="op", bufs=10) as opool,
            tc.tile_pool(name="ps", bufs=4, space="PSUM") as pspool,
        ):
            g0 = gpool.tile([L, 128], FP32, tag="g0")
            g1 = gpool.tile([128, 128], FP32, tag="g1")
            gt = gpool.tile([2 + LT, LT], FP32, tag="gt")
            nc.scalar.dma_start(out=g0[:], in_=g0_d)
            nc.scalar.dma_start(out=g1[:], in_=g1_d)
            nc.scalar.dma_start(out=gt[:], in_=gt_d)

            # round-0 inputs: rows 0..L-1 (no carry rows); split the issue
            # across both HWDGE engines to halve prologue latency
            xcur = []
            for g in range(NG):
                xs = xpool.tile([L, W], FP32, tag="x")
                for bb in range(GB):
                    b = g * GB + bb
                    eng = nc.sync if (g * GB + bb) % 2 == 0 else nc.scalar
                    eng.dma_start(
                        out=xs[0:L, bb * C:(bb + 1) * C], in_=x_d[b, 0:L, :]
                    )
                xcur.append(xs)

            ncopy = 0
            for i in range(NFULL + 1):
                for g in range(NG):
                    xs = xcur[g]
                    ps = pspool.tile([128, W], FP32, tag="ps")
                    if i < NFULL:
                        # next round's input tile for this group; issue the
                        # prefetch DMAs before this round's matmuls so the
                        # DMA queues never starve
                        if i + 1 < NFULL:
                            nxt = xpool.tile([128, W], FP32, tag="x")
                            nrows = L
                        else:
                            nxt = xpool.tile([2 + LT, W], FP32, tag="x")
                            nrows = LT
                        for bb in range(GB):
                            b = g * GB + bb
                            nc.sync.dma_start(
                                out=nxt[2:2 + nrows, bb * C:(bb + 1) * C],
                                in_=x_d[b, L * (i + 1):L * (i + 1) + nrows, :],
                            )
                        for bb in range(GB):
                            sl = slice(bb * C, (bb + 1) * C)
                            if i == 0:
                                nc.tensor.matmul(
                                    ps[:, sl], g0[:], xs[0:L, sl],
                                    start=True, stop=True,
                                )
                            else:
                                nc.tensor.matmul(
                                    ps[:, sl], g1[:], xs[:, sl],
                                    start=True, stop=True,
                                )
                        # chunk-end states -> next rhs rows 0-1
                        nc.vector.tensor_copy(out=nxt[0:2, :], in_=ps[0:2, :])
                        xcur[g] = nxt
                        o = opool.tile([128, W], FP32, tag="o")
                        nc.scalar.copy(out=o[:], in_=ps[:])
                        for bb in range(GB):
                            b = g * GB + bb
                            # near the kernel tail, spread out-DMA issue over
                            # the otherwise-idle DMA engines
                            if i >= NFULL - 1:
                                eng = (nc.gpsimd, nc.sync)[(g * GB + bb) % 2]
                            else:
                                eng = nc.gpsimd
                            eng.dma_start(
                                out=y_d[b, L * i:L * (i + 1), :],
                                in_=o[2:128, bb * C:(bb + 1) * C],
                            )
                    else:  # tail chunk (32 steps, no state outputs)
                        for bb in range(GB):
                            sl = slice(bb * C, (bb + 1) * C)
                            nc.tensor.matmul(
                                ps[0:LT, sl], gt[:], xs[0:2 + LT, sl],
                                start=True, stop=True,
                            )
                        o = opool.tile([LT, W], FP32, tag="o")
                        nc.scalar.copy(out=o[:], in_=ps[0:LT, :])
                        for bb in range(GB):
                            b = g * GB + bb
                            eng = (nc.gpsimd, nc.sync, nc.scalar)[(g * GB + bb) % 3]
                            eng.dma_start(
                                out=y_d[b, L * NFULL:T, :],
                                in_=o[:, bb * C:(bb + 1) * C],
                            )
    nc.compile()
    return nc


def _get_program():
    if "nc" not in _cache:
        _cache["nc"] = _build_program()
    return _cache["nc"]


def _ensure_axon_hooks_shim():
    """concourse's trace path does `from antenv.axon_hooks import ...`;
    some images lack that module. Install a no-op shim so an externally
    set BASS_TRACE can't crash the run (tracing then degrades to off)."""
    import types

    try:
        import antenv.axon_hooks  # noqa: F401
        return
    except ImportError:
        pass
    try:
        import antenv
    except ImportError:
        return
    mod = types.ModuleType("antenv.axon_hooks")
    mod.get_axon_ntff_profile_hook = lambda: None
    mod.set_axon_ntff_profile_hook = lambda h: None
    mod._kernel_shim = True
    sys.modules["antenv.axon_hooks"] = mod
    antenv.axon_hooks = mod


def _run(x, alpha, beta, trace=False):
    _ensure_axon_hooks_shim()
    from concourse.bass_utils import run_bass_kernel_spmd

    x = np.ascontiguousarray(np.asarray(x, dtype=np.float32))
    G0, G1, Gt = _build_mats(alpha, beta)
    nc = _get_program()
    in_maps = [
        {"x": x[c * BPC:(c + 1) * BPC], "g0": G0, "g1": G1, "gt": Gt}
        for c in range(NCORES)
    ]
    res = run_bass_kernel_spmd(nc, in_maps, list(range(NCORES)), trace=trace)
    out = np.concatenate([res.results[c]["y"] for c in range(NCORES)], axis=0)
    return out, res


def kernel(**inputs):
    alpha = float(np.asarray(inputs["alpha"]))
    beta = float(np.asarray(inputs["beta"]))
    out, _ = _run(inputs["x"], alpha, beta, trace=False)
    return out

